# revision 4
# baseline (speedup 1.0000x reference)
"""Causal self-attention (q/k-swapped variant) Bass kernel for Trainium2.

Problem: B=2, T=2048, C=768, H=12, hs=64.
    k = x@Wk+bk ; q = x@Wq+bq ; v = x@Wv+bv          (per-head split)
    att[b,h,i,j] = (k[b,i,h,:] . q[b,j,h,:]) / 8     (note: k rows, q cols)
    att = softmax(causal-mask(att), axis=j)
    y = (att @ v) @ Wo + bo

Sharding: 8 cores = 2 batches x 4 head-groups (3 heads each).
Each core computes its 3 heads fully (QKV proj -> attention -> partial
output projection); host sums the 4 partial outputs per batch and adds bo.

All on-device score math is done in "transposed score" space: score tiles
have j (softmax axis) on partitions and i on the free dim, so the PV matmul
needs no transposes at all, and the softmax denominator falls out of the PV
matmul via an appended ones-column on V.
"""

import os
import sys

sys.path.insert(0, "/opt/trn_rl_repo")

import numpy as np

T = 2048
C = 768
HS = 64
HPC = 3          # heads per core
NCH = C // 128   # 6 contraction chunks
TB = T // 128    # 16 row blocks
JB = T // 128    # 16 j blocks
NCORES = 8

_cache = {}


def _segments(lo, hi):
    """Split [lo, hi) at 512 boundaries (PSUM bank / fp32 matmul N limit)."""
    out = []
    s = lo
    while s < hi:
        e = min((s // 512 + 1) * 512, hi)
        out.append((s, e))
        s = e
    return out


def _emit(ctx, tc):
    import concourse.bass as bass
    import concourse.tile as tile  # noqa: F401
    from concourse import mybir
    from concourse.bass import ts
    from concourse.masks import make_upper_triangular

    f32 = mybir.dt.float32
    nc = tc.nc

    xT = nc.dram_tensor("xT", (C, T), f32, kind="ExternalInput").ap()
    wqk = nc.dram_tensor("wqk", (128, 3 * NCH * 128), f32, kind="ExternalInput").ap()
    wv = nc.dram_tensor("wv", (128, NCH * 192), f32, kind="ExternalInput").ap()
    wo = nc.dram_tensor("wo", (64, 3 * C), f32, kind="ExternalInput").ap()
    bqk = nc.dram_tensor("bqk", (1, 3 * 128), f32, kind="ExternalInput").ap()
    bv = nc.dram_tensor("bv", (1, 192), f32, kind="ExternalInput").ap()
    y = nc.dram_tensor("y", (T, C), f32, kind="ExternalOutput").ap()

    consts = ctx.enter_context(tc.tile_pool(name="consts", bufs=1))

    # ---- load inputs ----
    wqk_sb = consts.tile([128, 3, NCH, 128], f32)
    nc.sync.dma_start(wqk_sb[:], wqk.rearrange("p (g k m) -> p g k m", g=3, k=NCH))
    wv_sb = consts.tile([128, NCH, 192], f32)
    nc.sync.dma_start(wv_sb[:], wv.rearrange("p (k m) -> p k m", k=NCH))
    wo_sb = consts.tile([64, 3, C], f32)
    nc.sync.dma_start(wo_sb[:], wo.rearrange("p (h c) -> p h c", h=3))
    bqk_sb = consts.tile([1, 3, 128], f32)
    nc.sync.dma_start(bqk_sb[:], bqk.rearrange("p (g m) -> p g m", g=3))
    bv_sb = consts.tile([1, 192], f32)
    nc.sync.dma_start(bv_sb[:], bv)

    xT_sb = consts.tile([128, NCH, T], f32)
    for k in range(NCH):
        nc.sync.dma_start(xT_sb[:, k, :], xT[k * 128:(k + 1) * 128, :])

    ones_row = consts.tile([1, 512], f32)
    nc.vector.memset(ones_row[:], 1.0)
    ones_col = consts.tile([1, 128], f32)
    nc.vector.memset(ones_col[:], 1.0)
    trimask = consts.tile([128, 128], f32)
    make_upper_triangular(nc, trimask[:], val=1.0, diag=True)

    V_aug = consts.tile([128, TB, HPC * 65], f32)
    for h in range(HPC):
        nc.vector.memset(V_aug[:, :, h * 65 + 64:h * 65 + 65], 1.0)

    QK_sb = consts.tile([128, 3, T], f32)     # g0=Q(h0,h1) g1=K(h0,h1) g2=[Q(h2)|K(h2)]
    KT2_sb = consts.tile([64, T], f32)        # K(h2) shifted to base partition 0
    AT_sb = consts.tile([64, HPC, T], f32)    # normalized attn output, transposed

    # ---- phase 1: QKV projections ----
    with tc.tile_pool(name="psQK", bufs=2, space="PSUM") as psQK, \
         tc.tile_pool(name="psV", bufs=2, space="PSUM") as psV:
        for g in range(3):
            for it in range(T // 512):
                ps = psQK.tile([128, 512], f32)
                # bias as rank-1 matmul, then accumulate W^T x over C chunks
                nc.tensor.matmul(ps[:], lhsT=bqk_sb[:, g, :], rhs=ones_row[:],
                                 start=True, stop=False)
                for k in range(NCH):
                    nc.tensor.matmul(ps[:], lhsT=wqk_sb[:, g, k, :],
                                     rhs=xT_sb[:, k, ts(it, 512)],
                                     start=False, stop=(k == NCH - 1))
                nc.any.tensor_copy(QK_sb[:, g, ts(it, 512)], ps[:])
        for tb in range(TB):
            psv = psV.tile([128, 192], f32)
            nc.tensor.matmul(psv[:], lhsT=ones_col[:], rhs=bv_sb[:],
                             start=True, stop=False)
            for k in range(NCH):
                nc.tensor.matmul(psv[:], lhsT=xT_sb[:, k, ts(tb, 128)],
                                 rhs=wv_sb[:, k, :],
                                 start=False, stop=(k == NCH - 1))
            for h in range(HPC):
                nc.any.tensor_copy(V_aug[:, tb, h * 65:h * 65 + 64],
                                   psv[:, h * 64:(h + 1) * 64])

    # move K(h2) rows 64:128 -> partitions 0:64 so h2's ST operands line up
    nc.sync.dma_start(KT2_sb[:], QK_sb[64:128, 2, :])

    # per-head (lhsT=Q^T, rhs=K^T) access patterns; operand partition bases match
    heads = [
        (QK_sb[0:64, 0, :], QK_sb[0:64, 1, :]),
        (QK_sb[64:128, 0, :], QK_sb[64:128, 1, :]),
        (QK_sb[0:64, 2, :], KT2_sb[:, :]),
    ]

    # ---- phase 2: attention ----
    sbE = ctx.enter_context(tc.tile_pool(name="E", bufs=3))
    sbZ = ctx.enter_context(tc.tile_pool(name="Z", bufs=2))
    with tc.tile_pool(name="psS", bufs=2, space="PSUM") as psS, \
         tc.tile_pool(name="psO", bufs=1, space="PSUM") as psO:
        for h in range(HPC):
            QT, KT = heads[h]
            Vh = lambda jb: V_aug[:, jb, h * 65:(h + 1) * 65]  # noqa: E731
            Onum = psO.tile([65, T], f32)
            for jb in range(JB):
                i0 = 128 * jb
                for c in range(i0 // 1024, T // 1024):
                    lo = max(1024 * c, i0)
                    hi = 1024 * (c + 1)
                    S = psS.tile([128, 1024], f32)
                    for a, b in _segments(lo, hi):
                        nc.tensor.matmul(S[:, a - 1024 * c:b - 1024 * c],
                                         lhsT=QT[:, ts(jb, 128)],
                                         rhs=KT[:, a:b], start=True, stop=True)
                    E = sbE.tile([128, 1024], f32)
                    nc.scalar.activation(E[:, lo - 1024 * c:], S[:, lo - 1024 * c:],
                                         mybir.ActivationFunctionType.Exp,
                                         scale=0.125)
                    if lo == i0:  # chunk containing the diagonal block
                        r = i0 - 1024 * c
                        nc.vector.tensor_mul(E[:, r:r + 128], E[:, r:r + 128],
                                             trimask[:])
                    for a, b in _segments(lo, hi):
                        nc.tensor.matmul(Onum[:, a:b], lhsT=Vh(jb),
                                         rhs=E[:, a - 1024 * c:b - 1024 * c],
                                         start=(jb == 0),
                                         stop=(jb == 4 * (a // 512) + 3),
                                         skip_group_check=True)
            # normalize: row 64 of Onum is the softmax denominator
            rz = sbZ.tile([1, T], f32)
            nc.vector.reciprocal(rz[:], Onum[64:65, :])
            for m in range(T // 512):
                rzb = psS.tile([128, 1024], f32, tag="S")
                nc.tensor.matmul(rzb[0:64, 0:512], lhsT=ones_col[:, 0:64],
                                 rhs=rz[:, ts(m, 512)], start=True, stop=True)
                nc.any.tensor_copy(AT_sb[:, h, ts(m, 512)], Onum[0:64, ts(m, 512)])
                nc.vector.tensor_mul(AT_sb[:, h, ts(m, 512)],
                                     AT_sb[:, h, ts(m, 512)], rzb[0:64, 0:512])

    # ---- phase 3: output projection ----
    sbY = ctx.enter_context(tc.tile_pool(name="Y", bufs=2))
    with tc.tile_pool(name="psY", bufs=4, space="PSUM") as psY:
        for tb in range(TB):
            ysb = sbY.tile([128, C], f32)
            for n in range(2):
                psy = psY.tile([128, 384], f32)
                for h in range(HPC):
                    nc.tensor.matmul(psy[:], lhsT=AT_sb[:, h, ts(tb, 128)],
                                     rhs=wo_sb[:, h, ts(n, 384)],
                                     start=(h == 0), stop=(h == HPC - 1))
                nc.any.tensor_copy(ysb[:, ts(n, 384)], psy[:])
            nc.sync.dma_start(y[tb * 128:(tb + 1) * 128, :], ysb[:])


def _build():
    if "nc" in _cache:
        return _cache["nc"]
    from contextlib import ExitStack

    import concourse.tile as tile
    from concourse import bacc

    nc = bacc.Bacc("TRN2", target_bir_lowering=False, debug=False,
                   num_devices=NCORES)
    with tile.TileContext(nc) as tc:
        with ExitStack() as ctx:
            _emit(ctx, tc)
    nc.compile()
    _cache["nc"] = nc
    return nc


def _install_trace_hooks():
    """Make trace=True work in this container: shim the missing
    antenv.axon_hooks NTFF-profile hook (ctypes into libaxon_pjrt.so) and
    skip the S3 artifact upload."""
    import contextlib
    import ctypes
    import types

    import concourse.bass_utils as bu

    bu.upload_artifacts = lambda tmpdir: tmpdir
    try:
        from antenv.axon_hooks import get_axon_ntff_profile_hook  # noqa: F401
        return
    except ImportError:
        pass

    so_path = "/opt/axon/libaxon_pjrt.so"
    if not os.path.exists(so_path):
        return
    lib = ctypes.CDLL(so_path)
    if not hasattr(lib, "axon_start_nrt_profile"):
        return
    lib.axon_start_nrt_profile.argtypes = [
        ctypes.POINTER(ctypes.c_int64), ctypes.c_size_t,
    ]
    lib.axon_start_nrt_profile.restype = ctypes.c_int64
    lib.axon_stop_nrt_profile.argtypes = [ctypes.c_char_p]
    lib.axon_stop_nrt_profile.restype = ctypes.c_int64

    @contextlib.contextmanager
    def _hook(output_dir, device_ids):
        import jax
        jax.devices()
        if device_ids:
            ids = (ctypes.c_int64 * len(device_ids))(*device_ids)
            rc = lib.axon_start_nrt_profile(ids, len(device_ids))
        else:
            rc = lib.axon_start_nrt_profile(None, 0)
        if rc != 0:
            raise RuntimeError(f"axon_start_nrt_profile rc={rc}")
        try:
            yield
        finally:
            n = lib.axon_stop_nrt_profile(str(output_dir).encode())
            print(f"profile: {n} file(s) written to {output_dir}",
                  file=sys.stderr)

    state = {"h": _hook}
    mod = types.ModuleType("antenv.axon_hooks")
    mod.get_axon_ntff_profile_hook = lambda: state["h"]
    mod.set_axon_ntff_profile_hook = lambda h: state.__setitem__("h", h)
    import antenv
    antenv.axon_hooks = mod
    sys.modules["antenv.axon_hooks"] = mod


def kernel(**inputs):
    x = np.ascontiguousarray(np.asarray(inputs["x"], dtype=np.float32))
    Wq = np.asarray(inputs["Wq"], dtype=np.float32)
    Wk = np.asarray(inputs["Wk"], dtype=np.float32)
    Wv = np.asarray(inputs["Wv"], dtype=np.float32)
    Wo = np.asarray(inputs["Wo"], dtype=np.float32)
    bq = np.asarray(inputs["bq"], dtype=np.float32)
    bk = np.asarray(inputs["bk"], dtype=np.float32)
    bv = np.asarray(inputs["bv"], dtype=np.float32)
    bo = np.asarray(inputs["bo"], dtype=np.float32)

    from concourse import bass_utils

    nc = _build()

    B = x.shape[0]
    xTs = [np.ascontiguousarray(x[b].T) for b in range(B)]
    in_maps = []
    for core in range(NCORES):
        b, hg = core // 4, core % 4
        sl = slice(hg * 192, (hg + 1) * 192)
        wq_s, wk_s = Wq[:, sl], Wk[:, sl]
        g0 = wq_s[:, 0:128]
        g1 = wk_s[:, 0:128]
        g2 = np.concatenate([wq_s[:, 128:192], wk_s[:, 128:192]], axis=1)
        wqk_h = (np.stack([g0, g1, g2], 0)
                 .reshape(3, NCH, 128, 128).transpose(2, 0, 1, 3)
                 .reshape(128, 3 * NCH * 128))
        wv_h = (Wv[:, sl].reshape(NCH, 128, 192).transpose(1, 0, 2)
                .reshape(128, NCH * 192))
        wo_h = (Wo[sl, :].reshape(3, 64, C).transpose(1, 0, 2)
                .reshape(64, 3 * C))
        bqk_h = np.concatenate(
            [bq[sl][0:128], bk[sl][0:128], bq[sl][128:192], bk[sl][128:192]]
        ).reshape(1, 384)
        bv_h = bv[sl].reshape(1, 192)
        in_maps.append({
            "xT": xTs[b],
            "wqk": np.ascontiguousarray(wqk_h),
            "wv": np.ascontiguousarray(wv_h),
            "wo": np.ascontiguousarray(wo_h),
            "bqk": np.ascontiguousarray(bqk_h),
            "bv": np.ascontiguousarray(bv_h),
        })

    trace = bool(os.environ.get("KERNEL_TRACE"))
    if trace:
        _install_trace_hooks()
    res = bass_utils.run_bass_kernel_spmd(
        nc, in_maps, core_ids=list(range(NCORES)), trace=trace
    )
    _cache["last_results"] = res

    out = np.empty((B, T, C), dtype=np.float32)
    for b in range(B):
        acc = res.results[b * 4]["y"].copy()
        for hg in range(1, 4):
            acc += res.results[b * 4 + hg]["y"]
        out[b] = acc + bo
    return out


# revision 8
# speedup vs baseline: 1.0354x; 1.0354x over previous
"""Causal self-attention (q/k-swapped variant) Bass kernel for Trainium2.

Problem: B=2, T=2048, C=768, H=12, hs=64.
    k = x@Wk+bk ; q = x@Wq+bq ; v = x@Wv+bv          (per-head split)
    att[b,h,i,j] = (k[b,i,h,:] . q[b,j,h,:]) / 8     (note: k rows, q cols)
    att = softmax(causal-mask(att), axis=j)
    y = (att @ v) @ Wo + bo

Sharding: 8 cores = 2 batches x 4 head-groups (3 heads each).
Each core computes its 3 heads fully (QKV proj -> attention -> partial
output projection); host sums the 4 partial outputs per batch and adds bo.

All on-device score math is done in "transposed score" space: score tiles
have j (softmax axis) on partitions and i on the free dim, so the PV matmul
needs no transposes at all, and the softmax denominator falls out of the PV
matmul via an appended ones-column on V.
"""

import os
import sys

sys.path.insert(0, "/opt/trn_rl_repo")

import numpy as np

T = 2048
C = 768
HS = 64
HPC = 3          # heads per core
NCH = C // 128   # 6 contraction chunks
TB = T // 128    # 16 row blocks
JB = T // 128    # 16 j blocks
NCORES = 8

_cache = {}


def _segments(lo, hi):
    """Split [lo, hi) at 512 boundaries (PSUM bank / fp32 matmul N limit)."""
    out = []
    s = lo
    while s < hi:
        e = min((s // 512 + 1) * 512, hi)
        out.append((s, e))
        s = e
    return out


def _emit(ctx, tc):
    import concourse.bass as bass
    import concourse.tile as tile  # noqa: F401
    from concourse import mybir
    from concourse.bass import ts
    from concourse.masks import make_upper_triangular

    f32 = mybir.dt.float32
    nc = tc.nc

    xT = nc.dram_tensor("xT", (C, T), f32, kind="ExternalInput").ap()
    wqk = nc.dram_tensor("wqk", (128, 3 * NCH * 128), f32, kind="ExternalInput").ap()
    wv = nc.dram_tensor("wv", (128, NCH * 192), f32, kind="ExternalInput").ap()
    wo = nc.dram_tensor("wo", (64, 3 * C), f32, kind="ExternalInput").ap()
    bqk = nc.dram_tensor("bqk", (128, 3), f32, kind="ExternalInput").ap()
    bv = nc.dram_tensor("bv", (1, 192), f32, kind="ExternalInput").ap()
    y = nc.dram_tensor("y", (C, T), f32, kind="ExternalOutput").ap()  # transposed

    consts = ctx.enter_context(tc.tile_pool(name="consts", bufs=1))

    # ---- load inputs ----
    wqk_sb = consts.tile([128, 3, NCH, 128], f32)
    nc.sync.dma_start(wqk_sb[:], wqk.rearrange("p (g k m) -> p g k m", g=3, k=NCH))
    wv_sb = consts.tile([128, NCH, 192], f32)
    nc.sync.dma_start(wv_sb[:], wv.rearrange("p (k m) -> p k m", k=NCH))
    wo_sb = consts.tile([64, 3, C], f32)
    nc.sync.dma_start(wo_sb[:], wo.rearrange("p (h c) -> p h c", h=3))
    bqk_sb = consts.tile([128, 3], f32)       # per-partition bias per QK group
    nc.sync.dma_start(bqk_sb[:], bqk)
    bvb_sb = consts.tile([128, 192], f32)     # bv broadcast across partitions
    nc.sync.dma_start(bvb_sb[:], bv.to_broadcast((128, 192)))

    xT_sb = consts.tile([128, NCH, T], f32)
    for k in range(NCH):
        nc.sync.dma_start(xT_sb[:, k, :], xT[k * 128:(k + 1) * 128, :])

    ones_row = consts.tile([1, 512], f32)
    nc.vector.memset(ones_row[:], 1.0)
    ones_col = consts.tile([1, 128], f32)
    nc.vector.memset(ones_col[:], 1.0)
    trimask = consts.tile([128, 128], f32)
    make_upper_triangular(nc, trimask[:], val=1.0, diag=True)

    V_aug = consts.tile([128, TB, HPC * 65], f32)
    for h in range(HPC):
        nc.vector.memset(V_aug[:, :, h * 65 + 64:h * 65 + 65], 1.0)

    QK_sb = consts.tile([128, 3, T], f32)     # g0=Q(h0,h1) g1=K(h0,h1) g2=[Q(h2)|K(h2)]
    KT2_sb = consts.tile([64, T], f32)        # K(h2) shifted to base partition 0
    AT_sb = consts.tile([64, HPC, T], f32)    # normalized attn output, transposed

    # ---- phase 1: QKV projections ----
    with tc.tile_pool(name="psW", bufs=1, space="PSUM") as psW, \
         tc.tile_pool(name="psQK", bufs=2, space="PSUM") as psQK, \
         tc.tile_pool(name="psV", bufs=2, space="PSUM") as psV:
        # dummy matmuls: keep the PE busy (HAM warm) while inputs stream in
        warm = psW.tile([128, 512], f32, tag="warm")
        for _ in range(40):
            nc.tensor.matmul(warm[:], lhsT=ones_col[:], rhs=ones_row[:],
                             start=True, stop=True, skip_group_check=True)
        for g in range(3):
            for it in range(T // 512):
                ps = psQK.tile([128, 512], f32)
                for k in range(NCH):
                    nc.tensor.matmul(ps[:], lhsT=wqk_sb[:, g, k, :],
                                     rhs=xT_sb[:, k, ts(it, 512)],
                                     start=(k == 0), stop=(k == NCH - 1))
                nc.vector.tensor_add(QK_sb[:, g, ts(it, 512)], ps[:],
                                     bqk_sb[:, g:g + 1].to_broadcast((128, 512)))
        for tb in range(TB):
            psv = psV.tile([128, 192], f32)
            for k in range(NCH):
                nc.tensor.matmul(psv[:], lhsT=xT_sb[:, k, ts(tb, 128)],
                                 rhs=wv_sb[:, k, :],
                                 start=(k == 0), stop=(k == NCH - 1))
            for h in range(HPC):
                nc.any.tensor_add(V_aug[:, tb, h * 65:h * 65 + 64],
                                  psv[:, h * 64:(h + 1) * 64],
                                  bvb_sb[:, h * 64:(h + 1) * 64])

    # move K(h2) rows 64:128 -> partitions 0:64 so h2's ST operands line up
    nc.sync.dma_start(KT2_sb[:], QK_sb[64:128, 2, :])

    # per-head (lhsT=Q^T, rhs=K^T) access patterns; operand partition bases match
    heads = [
        (QK_sb[0:64, 0, :], QK_sb[0:64, 1, :]),
        (QK_sb[64:128, 0, :], QK_sb[64:128, 1, :]),
        (QK_sb[0:64, 2, :], KT2_sb[:, :]),
    ]

    # ---- phase 2: attention ----
    sbE = ctx.enter_context(tc.tile_pool(name="E", bufs=3))
    sbZ = ctx.enter_context(tc.tile_pool(name="Z", bufs=2))
    sbRZ = ctx.enter_context(tc.tile_pool(name="RZ", bufs=2))
    dramZ = ctx.enter_context(tc.tile_pool(name="dramZ", bufs=2, space="DRAM"))
    with tc.tile_pool(name="psS", bufs=2, space="PSUM") as psS, \
         tc.tile_pool(name="psO", bufs=1, space="PSUM") as psO:
        for h in range(HPC):
            QT, KT = heads[h]
            Vh = lambda jb: V_aug[:, jb, h * 65:(h + 1) * 65]  # noqa: E731
            Onum = psO.tile([65, T], f32)
            for jb in range(JB):
                i0 = 128 * jb
                for c in range(i0 // 1024, T // 1024):
                    lo = max(1024 * c, i0)
                    hi = 1024 * (c + 1)
                    S = psS.tile([128, 1024], f32)
                    for a, b in _segments(lo, hi):
                        nc.tensor.matmul(S[:, a - 1024 * c:b - 1024 * c],
                                         lhsT=QT[:, ts(jb, 128)],
                                         rhs=KT[:, a:b], start=True, stop=True)
                    E = sbE.tile([128, 1024], f32)
                    nc.scalar.activation(E[:, lo - 1024 * c:], S[:, lo - 1024 * c:],
                                         mybir.ActivationFunctionType.Exp,
                                         scale=0.125)
                    if lo == i0:  # chunk containing the diagonal block
                        r = i0 - 1024 * c
                        nc.vector.tensor_mul(E[:, r:r + 128], E[:, r:r + 128],
                                             trimask[:])
                    for a, b in _segments(lo, hi):
                        nc.tensor.matmul(Onum[:, a:b], lhsT=Vh(jb),
                                         rhs=E[:, a - 1024 * c:b - 1024 * c],
                                         start=(jb == 0),
                                         stop=(jb == 4 * (a // 512) + 3),
                                         skip_group_check=True)
            # normalize: row 64 of Onum is the softmax denominator Z.
            # reciprocal of a [1, T] row on one DVE lane is ~13us, so bounce
            # Z through DRAM to reshape it [128, T/128], recip there, bounce
            # back and DMA-broadcast to 64 partitions.
            zcol = sbZ.tile([65, T], f32)
            nc.scalar.copy(zcol[64:65, :], Onum[64:65, :])
            zd = dramZ.tile([1, T], f32)
            nc.sync.dma_start(zd[:], zcol[64:65, :])
            z16 = sbRZ.tile([128, T // 128], f32, tag="z16")
            nc.sync.dma_start(z16[:], zd[:].rearrange("o (p f) -> (o p) f", p=128))
            r16 = sbRZ.tile([128, T // 128], f32, tag="r16")
            nc.vector.reciprocal(r16[:], z16[:])
            rzd = dramZ.tile([1, T], f32, tag="rzd")
            nc.sync.dma_start(rzd[:].rearrange("o (p f) -> (o p) f", p=128), r16[:])
            rzb = sbRZ.tile([64, T], f32, tag="rzb")
            nc.sync.dma_start(rzb[:], rzd[:].to_broadcast((64, T)))
            nc.vector.tensor_mul(AT_sb[:, h, :], Onum[0:64, :], rzb[:])

    # ---- phase 3: output projection (in y^T layout: [C, T]) ----
    sbY = ctx.enter_context(tc.tile_pool(name="Y", bufs=2))
    with tc.tile_pool(name="psY", bufs=4, space="PSUM") as psY:
        for cb in range(NCH):
            ysb = sbY.tile([128, T], f32)
            for tt in range(T // 512):
                psy = psY.tile([128, 512], f32)
                for h in range(HPC):
                    nc.tensor.matmul(psy[:], lhsT=wo_sb[:, h, ts(cb, 128)],
                                     rhs=AT_sb[:, h, ts(tt, 512)],
                                     start=(h == 0), stop=(h == HPC - 1))
                nc.any.tensor_copy(ysb[:, ts(tt, 512)], psy[:])
            nc.sync.dma_start(y[cb * 128:(cb + 1) * 128, :], ysb[:])


def _build():
    if "nc" in _cache:
        return _cache["nc"]
    from contextlib import ExitStack

    import concourse.tile as tile
    from concourse import bacc

    nc = bacc.Bacc("TRN2", target_bir_lowering=False, debug=False,
                   num_devices=NCORES)
    with tile.TileContext(nc) as tc:
        with ExitStack() as ctx:
            _emit(ctx, tc)
    nc.compile()
    _cache["nc"] = nc
    return nc


def _install_trace_hooks():
    """Make trace=True work in this container: shim the missing
    antenv.axon_hooks NTFF-profile hook (ctypes into libaxon_pjrt.so) and
    skip the S3 artifact upload."""
    import contextlib
    import ctypes
    import types

    import concourse.bass_utils as bu

    bu.upload_artifacts = lambda tmpdir: tmpdir
    try:
        from antenv.axon_hooks import get_axon_ntff_profile_hook  # noqa: F401
        return
    except ImportError:
        pass

    so_path = "/opt/axon/libaxon_pjrt.so"
    if not os.path.exists(so_path):
        return
    lib = ctypes.CDLL(so_path)
    if not hasattr(lib, "axon_start_nrt_profile"):
        return
    lib.axon_start_nrt_profile.argtypes = [
        ctypes.POINTER(ctypes.c_int64), ctypes.c_size_t,
    ]
    lib.axon_start_nrt_profile.restype = ctypes.c_int64
    lib.axon_stop_nrt_profile.argtypes = [ctypes.c_char_p]
    lib.axon_stop_nrt_profile.restype = ctypes.c_int64

    @contextlib.contextmanager
    def _hook(output_dir, device_ids):
        import jax
        jax.devices()
        if device_ids:
            ids = (ctypes.c_int64 * len(device_ids))(*device_ids)
            rc = lib.axon_start_nrt_profile(ids, len(device_ids))
        else:
            rc = lib.axon_start_nrt_profile(None, 0)
        if rc != 0:
            raise RuntimeError(f"axon_start_nrt_profile rc={rc}")
        try:
            yield
        finally:
            n = lib.axon_stop_nrt_profile(str(output_dir).encode())
            print(f"profile: {n} file(s) written to {output_dir}",
                  file=sys.stderr)

    state = {"h": _hook}
    mod = types.ModuleType("antenv.axon_hooks")
    mod.get_axon_ntff_profile_hook = lambda: state["h"]
    mod.set_axon_ntff_profile_hook = lambda h: state.__setitem__("h", h)
    import antenv
    antenv.axon_hooks = mod
    sys.modules["antenv.axon_hooks"] = mod


def kernel(**inputs):
    x = np.ascontiguousarray(np.asarray(inputs["x"], dtype=np.float32))
    Wq = np.asarray(inputs["Wq"], dtype=np.float32)
    Wk = np.asarray(inputs["Wk"], dtype=np.float32)
    Wv = np.asarray(inputs["Wv"], dtype=np.float32)
    Wo = np.asarray(inputs["Wo"], dtype=np.float32)
    bq = np.asarray(inputs["bq"], dtype=np.float32)
    bk = np.asarray(inputs["bk"], dtype=np.float32)
    bv = np.asarray(inputs["bv"], dtype=np.float32)
    bo = np.asarray(inputs["bo"], dtype=np.float32)

    from concourse import bass_utils

    nc = _build()

    B = x.shape[0]
    xTs = [np.ascontiguousarray(x[b].T) for b in range(B)]
    in_maps = []
    for core in range(NCORES):
        b, hg = core // 4, core % 4
        sl = slice(hg * 192, (hg + 1) * 192)
        wq_s, wk_s = Wq[:, sl], Wk[:, sl]
        g0 = wq_s[:, 0:128]
        g1 = wk_s[:, 0:128]
        g2 = np.concatenate([wq_s[:, 128:192], wk_s[:, 128:192]], axis=1)
        wqk_h = (np.stack([g0, g1, g2], 0)
                 .reshape(3, NCH, 128, 128).transpose(2, 0, 1, 3)
                 .reshape(128, 3 * NCH * 128))
        wv_h = (Wv[:, sl].reshape(NCH, 128, 192).transpose(1, 0, 2)
                .reshape(128, NCH * 192))
        wo_h = (Wo[sl, :].reshape(3, 64, C).transpose(1, 0, 2)
                .reshape(64, 3 * C))
        bqk_h = np.stack(
            [bq[sl][0:128], bk[sl][0:128],
             np.concatenate([bq[sl][128:192], bk[sl][128:192]])], axis=1
        )  # [128, 3]
        bv_h = bv[sl].reshape(1, 192)
        in_maps.append({
            "xT": xTs[b],
            "wqk": np.ascontiguousarray(wqk_h),
            "wv": np.ascontiguousarray(wv_h),
            "wo": np.ascontiguousarray(wo_h),
            "bqk": np.ascontiguousarray(bqk_h),
            "bv": np.ascontiguousarray(bv_h),
        })

    trace = bool(os.environ.get("KERNEL_TRACE"))
    if trace:
        _install_trace_hooks()
    res = bass_utils.run_bass_kernel_spmd(
        nc, in_maps, core_ids=list(range(NCORES)), trace=trace
    )
    _cache["last_results"] = res

    out = np.empty((B, T, C), dtype=np.float32)
    for b in range(B):
        acc = res.results[b * 4]["y"].copy()
        for hg in range(1, 4):
            acc += res.results[b * 4 + hg]["y"]
        out[b] = acc.T + bo
    return out


# revision 12
# speedup vs baseline: 2.3522x; 2.2718x over previous
"""Causal self-attention (q/k-swapped variant) Bass kernel for Trainium2.

Problem: B=2, T=2048, C=768, H=12, hs=64.
    k = x@Wk+bk ; q = x@Wq+bq ; v = x@Wv+bv          (per-head split)
    att[b,h,i,j] = (k[b,i,h,:] . q[b,j,h,:]) / 8     (note: k rows, q cols)
    att = softmax(causal-mask(att), axis=j)
    y = (att @ v) @ Wo + bo

Sharding: 8 cores = 2 batches x 4 head-groups (3 heads each).
Each core computes its 3 heads fully (QKV proj -> attention -> partial
output projection); host sums the 4 partial outputs per batch and adds bo.

All on-device score math is done in "transposed score" space: score tiles
have j (softmax axis) on partitions and i on the free dim, so the PV matmul
needs no transposes at all, and the softmax denominator falls out of the PV
matmul via an appended ones-column on V.
"""

import os
import sys

sys.path.insert(0, "/opt/trn_rl_repo")

import numpy as np

T = 2048
C = 768
HS = 64
HPC = 3          # heads per core
NCH = C // 128   # 6 contraction chunks
TB = T // 128    # 16 row blocks
JB = T // 128    # 16 j blocks
NCORES = 8
USE_BF16 = os.environ.get("KERNEL_FP32") is None  # matmul inputs in bf16

_cache = {}


def _segments(lo, hi):
    """Split [lo, hi) at 512 boundaries (PSUM bank / fp32 matmul N limit)."""
    out = []
    s = lo
    while s < hi:
        e = min((s // 512 + 1) * 512, hi)
        out.append((s, e))
        s = e
    return out


def _emit(ctx, tc):
    import concourse.bass as bass
    import concourse.tile as tile  # noqa: F401
    from concourse import mybir
    from concourse.bass import ts
    from concourse.masks import make_upper_triangular

    f32 = mybir.dt.float32
    mmd = mybir.dt.bfloat16 if USE_BF16 else f32  # matmul-input dtype
    nc = tc.nc

    xT = nc.dram_tensor("xT", (C, T), mmd, kind="ExternalInput").ap()
    wqk = nc.dram_tensor("wqk", (128, 3 * NCH * 128), mmd, kind="ExternalInput").ap()
    wv = nc.dram_tensor("wv", (128, NCH * 192), mmd, kind="ExternalInput").ap()
    wo = nc.dram_tensor("wo", (64, 3 * C), mmd, kind="ExternalInput").ap()
    bqk = nc.dram_tensor("bqk", (128, 3), f32, kind="ExternalInput").ap()
    bv = nc.dram_tensor("bv", (1, 192), f32, kind="ExternalInput").ap()
    y = nc.dram_tensor("y", (C, T), f32, kind="ExternalOutput").ap()  # transposed

    consts = ctx.enter_context(tc.tile_pool(name="consts", bufs=1))

    # ---- load inputs ----
    wqk_sb = consts.tile([128, 3, NCH, 128], mmd)
    nc.sync.dma_start(wqk_sb[:], wqk.rearrange("p (g k m) -> p g k m", g=3, k=NCH))
    wv_sb = consts.tile([128, NCH, 192], mmd)
    nc.sync.dma_start(wv_sb[:], wv.rearrange("p (k m) -> p k m", k=NCH))
    wo_sb = consts.tile([64, 3, C], mmd)
    nc.sync.dma_start(wo_sb[:], wo.rearrange("p (h c) -> p h c", h=3))
    bqk_sb = consts.tile([128, 3], f32)       # per-partition bias per QK group
    nc.sync.dma_start(bqk_sb[:], bqk)
    bvb_sb = consts.tile([128, 192], f32)     # bv broadcast across partitions
    nc.sync.dma_start(bvb_sb[:], bv.to_broadcast((128, 192)))

    xT_sb = consts.tile([128, NCH, T], mmd)
    for k in range(NCH):
        nc.sync.dma_start(xT_sb[:, k, :], xT[k * 128:(k + 1) * 128, :])

    scratch = consts.tile([128, 512], mmd)
    nc.vector.memset(scratch[:], 0.0)
    trimask = consts.tile([128, 128], mmd)
    make_upper_triangular(nc, trimask[:], val=1.0, diag=True)

    V_aug = consts.tile([128, TB, HPC * 65], mmd)
    for h in range(HPC):
        nc.vector.memset(V_aug[:, :, h * 65 + 64:h * 65 + 65], 1.0)

    QK_sb = consts.tile([128, 3, T], mmd)     # g0=Q(h0,h1) g1=K(h0,h1) g2=[Q(h2)|K(h2)]
    KT2_sb = consts.tile([64, T], mmd)        # K(h2) shifted to base partition 0
    AT_sb = consts.tile([64, HPC, T], mmd)    # normalized attn output, transposed

    # ---- phase 1: QKV projections ----
    with tc.tile_pool(name="psW", bufs=1, space="PSUM") as psW, \
         tc.tile_pool(name="psQK", bufs=2, space="PSUM") as psQK, \
         tc.tile_pool(name="psV", bufs=2, space="PSUM") as psV:
        # dummy matmuls: keep the PE busy (HAM warm) while inputs stream in
        warm = psW.tile([128, 512], f32, tag="warm")
        for _ in range(48):
            nc.tensor.matmul(warm[:], lhsT=scratch[:, 0:128], rhs=scratch[:],
                             start=True, stop=True, skip_group_check=True)
        for g in range(3):
            for it in range(T // 512):
                ps = psQK.tile([128, 512], f32)
                for k in range(NCH):
                    nc.tensor.matmul(ps[:], lhsT=wqk_sb[:, g, k, :],
                                     rhs=xT_sb[:, k, ts(it, 512)],
                                     start=(k == 0), stop=(k == NCH - 1))
                nc.vector.tensor_add(QK_sb[:, g, ts(it, 512)], ps[:],
                                     bqk_sb[:, g:g + 1].to_broadcast((128, 512)))
        for tb in range(TB):
            psv = psV.tile([128, 192], f32)
            for k in range(NCH):
                nc.tensor.matmul(psv[:], lhsT=xT_sb[:, k, ts(tb, 128)],
                                 rhs=wv_sb[:, k, :],
                                 start=(k == 0), stop=(k == NCH - 1))
            for h in range(HPC):
                nc.any.tensor_add(V_aug[:, tb, h * 65:h * 65 + 64],
                                  psv[:, h * 64:(h + 1) * 64],
                                  bvb_sb[:, h * 64:(h + 1) * 64])

    # move K(h2) rows 64:128 -> partitions 0:64 so h2's ST operands line up
    nc.sync.dma_start(KT2_sb[:], QK_sb[64:128, 2, :])

    # per-head (lhsT=Q^T, rhs=K^T) access patterns; operand partition bases match
    heads = [
        (QK_sb[0:64, 0, :], QK_sb[0:64, 1, :]),
        (QK_sb[64:128, 0, :], QK_sb[64:128, 1, :]),
        (QK_sb[0:64, 2, :], KT2_sb[:, :]),
    ]

    # ---- phase 2: attention ----
    sbE = ctx.enter_context(tc.tile_pool(name="E", bufs=3))
    sbZ = ctx.enter_context(tc.tile_pool(name="Z", bufs=2))
    sbRZ = ctx.enter_context(tc.tile_pool(name="RZ", bufs=2))
    dramZ = ctx.enter_context(tc.tile_pool(name="dramZ", bufs=2, space="DRAM"))
    with tc.tile_pool(name="psS", bufs=2, space="PSUM") as psS, \
         tc.tile_pool(name="psO", bufs=1, space="PSUM") as psO:
        for h in range(HPC):
            QT, KT = heads[h]
            Vh = lambda jb: V_aug[:, jb, h * 65:(h + 1) * 65]  # noqa: E731
            Onum = psO.tile([65, T], f32)
            for jb in range(JB):
                i0 = 128 * jb
                for c in range(i0 // 1024, T // 1024):
                    lo = max(1024 * c, i0)
                    hi = 1024 * (c + 1)
                    S = psS.tile([128, 1024], f32)
                    for a, b in _segments(lo, hi):
                        nc.tensor.matmul(S[:, a - 1024 * c:b - 1024 * c],
                                         lhsT=QT[:, ts(jb, 128)],
                                         rhs=KT[:, a:b], start=True, stop=True)
                    E = sbE.tile([128, 1024], mmd)
                    nc.scalar.activation(E[:, lo - 1024 * c:], S[:, lo - 1024 * c:],
                                         mybir.ActivationFunctionType.Exp,
                                         scale=0.125)
                    if lo == i0:  # chunk containing the diagonal block
                        r = i0 - 1024 * c
                        nc.vector.tensor_mul(E[:, r:r + 128], E[:, r:r + 128],
                                             trimask[:])
                    for a, b in _segments(lo, hi):
                        nc.tensor.matmul(Onum[:, a:b], lhsT=Vh(jb),
                                         rhs=E[:, a - 1024 * c:b - 1024 * c],
                                         start=(jb == 0),
                                         stop=(jb == 4 * (a // 512) + 3),
                                         skip_group_check=True)
            # normalize: row 64 of Onum is the softmax denominator Z.
            # reciprocal of a [1, T] row on one DVE lane is ~13us, so bounce
            # Z through DRAM to reshape it [128, T/128], recip there, bounce
            # back and DMA-broadcast to 64 partitions.
            zcol = sbZ.tile([65, T], f32)
            nc.scalar.copy(zcol[64:65, :], Onum[64:65, :])
            zd = dramZ.tile([1, T], f32)
            nc.sync.dma_start(zd[:], zcol[64:65, :])
            z16 = sbRZ.tile([128, T // 128], f32, tag="z16")
            nc.sync.dma_start(z16[:], zd[:].rearrange("o (p f) -> (o p) f", p=128))
            r16 = sbRZ.tile([128, T // 128], f32, tag="r16")
            nc.vector.reciprocal(r16[:], z16[:])
            rzd = dramZ.tile([1, T], f32, tag="rzd")
            nc.sync.dma_start(rzd[:].rearrange("o (p f) -> (o p) f", p=128), r16[:])
            rzb = sbRZ.tile([64, T], f32, tag="rzb")
            nc.sync.dma_start(rzb[:], rzd[:].to_broadcast((64, T)))
            nc.vector.tensor_mul(AT_sb[:, h, :], Onum[0:64, :], rzb[:])

    # ---- phase 3: output projection (in y^T layout: [C, T]) ----
    sbY = ctx.enter_context(tc.tile_pool(name="Y", bufs=2))
    with tc.tile_pool(name="psY", bufs=4, space="PSUM") as psY:
        for cb in range(NCH):
            ysb = sbY.tile([128, T], f32)
            for tt in range(T // 512):
                psy = psY.tile([128, 512], f32)
                for h in range(HPC):
                    nc.tensor.matmul(psy[:], lhsT=wo_sb[:, h, ts(cb, 128)],
                                     rhs=AT_sb[:, h, ts(tt, 512)],
                                     start=(h == 0), stop=(h == HPC - 1))
                nc.any.tensor_copy(ysb[:, ts(tt, 512)], psy[:])
            nc.sync.dma_start(y[cb * 128:(cb + 1) * 128, :], ysb[:])


def _build():
    if "nc" in _cache:
        return _cache["nc"]
    from contextlib import ExitStack

    import concourse.tile as tile
    from concourse import bacc

    nc = bacc.Bacc("TRN2", target_bir_lowering=False, debug=False,
                   num_devices=NCORES)
    with tile.TileContext(nc) as tc:
        with ExitStack() as ctx:
            _emit(ctx, tc)
    nc.compile()
    _cache["nc"] = nc
    return nc


def _install_trace_hooks():
    """Make trace=True work in this container: shim the missing
    antenv.axon_hooks NTFF-profile hook (ctypes into libaxon_pjrt.so) and
    skip the S3 artifact upload."""
    import contextlib
    import ctypes
    import types

    import concourse.bass_utils as bu

    bu.upload_artifacts = lambda tmpdir: tmpdir
    try:
        from antenv.axon_hooks import get_axon_ntff_profile_hook  # noqa: F401
        return
    except ImportError:
        pass

    so_path = "/opt/axon/libaxon_pjrt.so"
    if not os.path.exists(so_path):
        return
    lib = ctypes.CDLL(so_path)
    if not hasattr(lib, "axon_start_nrt_profile"):
        return
    lib.axon_start_nrt_profile.argtypes = [
        ctypes.POINTER(ctypes.c_int64), ctypes.c_size_t,
    ]
    lib.axon_start_nrt_profile.restype = ctypes.c_int64
    lib.axon_stop_nrt_profile.argtypes = [ctypes.c_char_p]
    lib.axon_stop_nrt_profile.restype = ctypes.c_int64

    @contextlib.contextmanager
    def _hook(output_dir, device_ids):
        import jax
        jax.devices()
        if device_ids:
            ids = (ctypes.c_int64 * len(device_ids))(*device_ids)
            rc = lib.axon_start_nrt_profile(ids, len(device_ids))
        else:
            rc = lib.axon_start_nrt_profile(None, 0)
        if rc != 0:
            raise RuntimeError(f"axon_start_nrt_profile rc={rc}")
        try:
            yield
        finally:
            n = lib.axon_stop_nrt_profile(str(output_dir).encode())
            print(f"profile: {n} file(s) written to {output_dir}",
                  file=sys.stderr)

    state = {"h": _hook}
    mod = types.ModuleType("antenv.axon_hooks")
    mod.get_axon_ntff_profile_hook = lambda: state["h"]
    mod.set_axon_ntff_profile_hook = lambda h: state.__setitem__("h", h)
    import antenv
    antenv.axon_hooks = mod
    sys.modules["antenv.axon_hooks"] = mod


def kernel(**inputs):
    x = np.ascontiguousarray(np.asarray(inputs["x"], dtype=np.float32))
    Wq = np.asarray(inputs["Wq"], dtype=np.float32)
    Wk = np.asarray(inputs["Wk"], dtype=np.float32)
    Wv = np.asarray(inputs["Wv"], dtype=np.float32)
    Wo = np.asarray(inputs["Wo"], dtype=np.float32)
    bq = np.asarray(inputs["bq"], dtype=np.float32)
    bk = np.asarray(inputs["bk"], dtype=np.float32)
    bv = np.asarray(inputs["bv"], dtype=np.float32)
    bo = np.asarray(inputs["bo"], dtype=np.float32)

    from concourse import bass_utils

    nc = _build()

    if USE_BF16:
        import ml_dtypes
        mmd_np = ml_dtypes.bfloat16
    else:
        mmd_np = np.float32

    B = x.shape[0]
    xTs = [np.ascontiguousarray(x[b].T.astype(mmd_np)) for b in range(B)]
    in_maps = []
    for core in range(NCORES):
        b, hg = core // 4, core % 4
        sl = slice(hg * 192, (hg + 1) * 192)
        wq_s, wk_s = Wq[:, sl], Wk[:, sl]
        g0 = wq_s[:, 0:128]
        g1 = wk_s[:, 0:128]
        g2 = np.concatenate([wq_s[:, 128:192], wk_s[:, 128:192]], axis=1)
        wqk_h = (np.stack([g0, g1, g2], 0)
                 .reshape(3, NCH, 128, 128).transpose(2, 0, 1, 3)
                 .reshape(128, 3 * NCH * 128))
        wv_h = (Wv[:, sl].reshape(NCH, 128, 192).transpose(1, 0, 2)
                .reshape(128, NCH * 192))
        wo_h = (Wo[sl, :].reshape(3, 64, C).transpose(1, 0, 2)
                .reshape(64, 3 * C))
        bqk_h = np.stack(
            [bq[sl][0:128], bk[sl][0:128],
             np.concatenate([bq[sl][128:192], bk[sl][128:192]])], axis=1
        )  # [128, 3]
        bv_h = bv[sl].reshape(1, 192)
        in_maps.append({
            "xT": xTs[b],
            "wqk": np.ascontiguousarray(wqk_h.astype(mmd_np)),
            "wv": np.ascontiguousarray(wv_h.astype(mmd_np)),
            "wo": np.ascontiguousarray(wo_h.astype(mmd_np)),
            "bqk": np.ascontiguousarray(bqk_h),
            "bv": np.ascontiguousarray(bv_h),
        })

    trace = bool(os.environ.get("KERNEL_TRACE"))
    if trace:
        _install_trace_hooks()
    res = bass_utils.run_bass_kernel_spmd(
        nc, in_maps, core_ids=list(range(NCORES)), trace=trace
    )
    _cache["last_results"] = res

    out = np.empty((B, T, C), dtype=np.float32)
    for b in range(B):
        acc = res.results[b * 4]["y"].copy()
        for hg in range(1, 4):
            acc += res.results[b * 4 + hg]["y"]
        out[b] = acc.T + bo
    return out
